# revision 4
# baseline (speedup 1.0000x reference)
"""AttnBlock (GroupNorm -> QKV 1x1 -> single-head attention over 4096 tokens
-> proj -> residual) on 8 Trainium2 NeuronCores, data-parallel over batch.

Per-core layout strategy (one image per core, N=4096 tokens, C=512 channels):
  - x loaded token-major, PE-transposed to channel-major xT; GroupNorm stats
    via ones-lhsT colsum matmuls in token-major; normalization fused into the
    transpose's PSUM->SBUF copy (DVE tensor_scalar with per-partition a, b).
  - q^T, k^T computed channel-major (lhsT = w tiles, rhs = hT); q^T spilled to
    DRAM and streamed back per query group; k^T kept resident.
  - v computed token-major (lhsT = hT tiles, rhs = wv), spilled to DRAM,
    streamed back per (group, j-tile).
  - Attention per query group of 512 queries: scores^T tiles [j=128, i=512]
    (lhsT = kT, rhs = qT slice), exp on ACT (scale = 1/sqrt(C)) directly into
    SBUF as float32r, flash-style accumulation: AV matmuls (lhsT = v tile,
    rhs = exp tile) and denominator matmuls (lhsT = ones column) accumulate
    in PSUM over the 32 j-tiles.  No max-subtraction: scores for this problem
    are O(5), exp is safe in f32.
  - proj: lhsT = AV^T tiles (copied PSUM->SBUF), rhs = wo; softmax
    normalization (1/denominator, transposed to per-partition via a DRAM
    bounce) and residual+bias applied in one DVE scalar_tensor_tensor.
All matmuls run in float32r (~tf32 precision, 1 cycle/row at N=512).
"""

import numpy as np

import concourse.bass as bass
import concourse.tile as tile
from concourse import bacc, mybir
from concourse.bass_utils import run_bass_kernel_spmd

B, H, W, C = 8, 64, 64, 512
N = H * W            # 4096 tokens per image
G = 32               # groups
EPS = 1e-5
N_CORES = 8

F32 = mybir.dt.float32
F32R = mybir.dt.float32r
AF = mybir.ActivationFunctionType
ALU = mybir.AluOpType

NT = N // 128        # 32 token tiles
CT = C // 128        # 4 channel tiles
NG = 8               # query groups
GW = N // NG         # 512 queries per group
NB = GW // 128       # 4 token blocks per group
CPG = C // G         # 16 channels per group


def build_program(reps: int = 1):
    nc = bacc.Bacc("TRN2", target_bir_lowering=False, debug=False,
                   num_devices=N_CORES)

    x_ap = nc.dram_tensor("x", [N, C], F32, kind="ExternalInput").ap()
    wq_ap = nc.dram_tensor("wq", [C, C], F32, kind="ExternalInput").ap()
    wk_ap = nc.dram_tensor("wk", [C, C], F32, kind="ExternalInput").ap()
    wv_ap = nc.dram_tensor("wv", [C, C], F32, kind="ExternalInput").ap()
    wo_ap = nc.dram_tensor("wo", [C, C], F32, kind="ExternalInput").ap()
    bq_ap = nc.dram_tensor("bq", [C], F32, kind="ExternalInput").ap()
    bk_ap = nc.dram_tensor("bk", [C], F32, kind="ExternalInput").ap()
    bv_ap = nc.dram_tensor("bv", [C], F32, kind="ExternalInput").ap()
    bo_ap = nc.dram_tensor("bo", [C], F32, kind="ExternalInput").ap()
    gns_ap = nc.dram_tensor("gn_scale", [C], F32, kind="ExternalInput").ap()
    gnb_ap = nc.dram_tensor("gn_bias", [C], F32, kind="ExternalInput").ap()
    id_ap = nc.dram_tensor("ident", [128, 128], F32, kind="ExternalInput").ap()
    out_ap = nc.dram_tensor("out", [N, C], F32, kind="ExternalOutput").ap()

    qt_dram = nc.dram_tensor("qt_scratch", [CT, 128, N], F32R).ap()
    v_dram = nc.dram_tensor("v_scratch", [NT, 128, C], F32R).ap()
    gn_bounce = nc.dram_tensor("gn_bounce", [2, C], F32).ap()
    den_bounce = nc.dram_tensor("den_bounce", [NG, GW], F32).ap()

    x_r = x_ap.rearrange("(nt p) c -> nt p c", p=128)
    out_r = out_ap.rearrange("(nt p) c -> nt p c", p=128)

    with tile.TileContext(nc) as tc, \
         nc.allow_low_precision(reason="float32r attention by design"):
        rep_ctx = tc.For_i(0, reps, 1) if reps > 1 else None
        import contextlib
        with contextlib.ExitStack() as st:
            if rep_ctx is not None:
                st.enter_context(rep_ctx)
            consts = st.enter_context(tc.tile_pool(name="consts", bufs=1))
            big = st.enter_context(tc.tile_pool(name="big", bufs=1))
            small = st.enter_context(tc.tile_pool(name="small", bufs=1))

            # ---- constants -------------------------------------------------
            id_raw = consts.tile([128, 128], F32, tag="id_raw")
            nc.sync.dma_start(id_raw[:], id_ap[:])
            id_r = consts.tile([128, 128], F32R, tag="id_r")
            nc.vector.tensor_copy(id_r[:], id_raw[:])
            ones_f = consts.tile([128, 1], F32, tag="ones_f")
            nc.vector.memset(ones_f[:], 1.0)
            ones_r = consts.tile([128, 1], F32R, tag="ones_r")
            nc.vector.tensor_copy(ones_r[:], ones_f[:])

            w_r = {}
            for name, ap in (("wq", wq_ap), ("wk", wk_ap), ("wv", wv_ap),
                             ("wo", wo_ap)):
                raw = small.tile([128, CT, C], F32, tag="w_raw")
                nc.sync.dma_start(raw[:], ap.rearrange("(ct p) d -> p ct d", p=128))
                wr = consts.tile([128, CT, C], F32R, tag=f"{name}_r")
                nc.vector.tensor_copy(wr[:], raw[:])
                w_r[name] = wr

            bqk_sb = consts.tile([128, 2, CT], F32, tag="bqk")
            nc.sync.dma_start(bqk_sb[:, 0, :], bq_ap.rearrange("(dt p) -> p dt", p=128))
            nc.sync.dma_start(bqk_sb[:, 1, :], bk_ap.rearrange("(dt p) -> p dt", p=128))
            bvb = consts.tile([128, C], F32, tag="bvb")
            nc.sync.dma_start(bvb[:], bv_ap.unsqueeze(0).partition_broadcast(128))
            bob = consts.tile([128, C], F32, tag="bob")
            nc.sync.dma_start(bob[:], bo_ap.unsqueeze(0).partition_broadcast(128))
            gns_sb = consts.tile([1, C], F32, tag="gns")
            nc.sync.dma_start(gns_sb[:], gns_ap.unsqueeze(0))
            gnb_sb = consts.tile([1, C], F32, tag="gnb")
            nc.sync.dma_start(gnb_sb[:], gnb_ap.unsqueeze(0))

            with tc.tile_pool(name="hTp", bufs=1) as hTp:
                hT = hTp.tile([128, CT, N], F32R, tag="hT")

                # ---- phase A: load x, stats, transpose + groupnorm ---------
                xr_all = big.tile([128, NT, C], F32R, tag="big")
                with (
                    tc.tile_pool(name="pa_ps", bufs=1, space=bass.MemorySpace.PSUM) as paps,
                    tc.tile_pool(name="pa_tps", bufs=4, space=bass.MemorySpace.PSUM) as patps,
                    tc.tile_pool(name="xin", bufs=3) as xin,
                    tc.tile_pool(name="x2p", bufs=2) as x2p,
                ):
                    s1_ps = paps.tile([1, C], F32, tag="s1")
                    s2_ps = paps.tile([1, C], F32, tag="s2")
                    for nt in range(NT):
                        x_t = xin.tile([128, C], F32, tag="x_t")
                        nc.sync.dma_start(x_t[:], x_r[nt])
                        xr_t = xr_all[:, nt, :]
                        nc.vector.tensor_copy(xr_t, x_t[:])
                        x2_t = x2p.tile([128, C], F32R, tag="x2_t")
                        nc.vector.tensor_tensor(x2_t[:], xr_t, xr_t, op=ALU.mult)
                        nc.tensor.matmul(s1_ps[:], ones_r[:], xr_t,
                                         start=(nt == 0), stop=(nt == NT - 1))
                        nc.tensor.matmul(s2_ps[:], ones_r[:], x2_t[:],
                                         start=(nt == 0), stop=(nt == NT - 1))

                    # group stats on partition 0
                    g1 = small.tile([1, G], F32, tag="g1")
                    nc.vector.reduce_sum(
                        g1[:], s1_ps[:].rearrange("p (g k) -> p g k", k=CPG),
                        axis=mybir.AxisListType.X)
                    g2 = small.tile([1, G], F32, tag="g2")
                    nc.vector.reduce_sum(
                        g2[:], s2_ps[:].rearrange("p (g k) -> p g k", k=CPG),
                        axis=mybir.AxisListType.X)
                    cnt = 1.0 / (N * CPG)
                    mean = small.tile([1, G], F32, tag="mean")
                    nc.scalar.mul(mean[:], g1[:], cnt)
                    ex2 = small.tile([1, G], F32, tag="ex2")
                    nc.scalar.mul(ex2[:], g2[:], cnt)
                    var = small.tile([1, G], F32, tag="var")
                    nc.vector.tensor_tensor(var[:], mean[:], mean[:], op=ALU.mult)
                    nc.vector.tensor_tensor(var[:], ex2[:], var[:], op=ALU.subtract)
                    eps_t = small.tile([1, 1], F32, tag="eps_t")
                    nc.vector.memset(eps_t[:], EPS)
                    sd = small.tile([1, G], F32, tag="sd")
                    nc.scalar.activation(sd[:], var[:], AF.Sqrt, bias=eps_t[:])
                    inv = small.tile([1, G], F32, tag="inv")
                    nc.vector.reciprocal(inv[:], sd[:])
                    # broadcast group -> channel (free-dim stride-0 read)
                    invc = small.tile([1, C], F32, tag="invc")
                    nc.vector.tensor_copy(
                        invc[:].rearrange("p (g k) -> p g k", k=CPG),
                        inv[:].unsqueeze(2).broadcast_to([1, G, CPG]))
                    meanc = small.tile([1, C], F32, tag="meanc")
                    nc.vector.tensor_copy(
                        meanc[:].rearrange("p (g k) -> p g k", k=CPG),
                        mean[:].unsqueeze(2).broadcast_to([1, G, CPG]))
                    a_c = small.tile([1, C], F32, tag="a_c")
                    nc.vector.tensor_tensor(a_c[:], invc[:], gns_sb[:], op=ALU.mult)
                    b_c = small.tile([1, C], F32, tag="b_c")
                    nc.vector.tensor_tensor(b_c[:], meanc[:], a_c[:], op=ALU.mult)
                    nc.vector.tensor_tensor(b_c[:], gnb_sb[:], b_c[:], op=ALU.subtract)
                    # bounce [1, C] -> per-partition [128, 2, CT]
                    nc.sync.dma_start(gn_bounce[0].unsqueeze(0), a_c[:])
                    nc.sync.dma_start(gn_bounce[1].unsqueeze(0), b_c[:])
                    ab_sb = small.tile([128, 2, CT], F32, tag="ab_sb")
                    nc.sync.dma_start(
                        ab_sb[:], gn_bounce.rearrange("two (ct p) -> p two ct", p=128))

                    # transpose + fused groupnorm apply
                    for nt in range(NT):
                        for ct in range(CT):
                            tp = patps.tile([128, 128], F32R, tag="tp")
                            nc.tensor.transpose(
                                tp[:], xr_all[:, nt, bass.ts(ct, 128)], id_r[:])
                            nc.vector.tensor_scalar(
                                hT[:, ct, bass.ts(nt, 128)], tp[:],
                                ab_sb[:, 0, ct:ct + 1], ab_sb[:, 1, ct:ct + 1],
                                op0=ALU.mult, op1=ALU.add)

                # ---- phase B: q^T, k^T, v ---------------------------------
                kT = big.tile([128, CT, N], F32R, tag="big")
                with (
                    tc.tile_pool(name="pb_ps", bufs=4, space=bass.MemorySpace.PSUM) as pbps,
                    tc.tile_pool(name="qko", bufs=3) as qko,
                ):
                    for dt in range(CT):
                        for nb in range(N // 512):
                            q_ps = pbps.tile([128, 512], F32, tag="qkv_ps")
                            for ct in range(CT):
                                nc.tensor.matmul(
                                    q_ps[:],
                                    w_r["wq"][:, ct, bass.ts(dt, 128)],
                                    hT[:, ct, bass.ts(nb, 512)],
                                    start=(ct == 0), stop=(ct == CT - 1))
                            q_sb = qko.tile([128, 512], F32R, tag="q_sb")
                            nc.scalar.activation(q_sb[:], q_ps[:], AF.Identity,
                                                 bias=bqk_sb[:, 0, dt:dt + 1])
                            nc.sync.dma_start(qt_dram[dt, :, bass.ts(nb, 512)], q_sb[:])

                            k_ps = pbps.tile([128, 512], F32, tag="qkv_ps")
                            for ct in range(CT):
                                nc.tensor.matmul(
                                    k_ps[:],
                                    w_r["wk"][:, ct, bass.ts(dt, 128)],
                                    hT[:, ct, bass.ts(nb, 512)],
                                    start=(ct == 0), stop=(ct == CT - 1))
                            nc.scalar.activation(kT[:, dt, bass.ts(nb, 512)], k_ps[:],
                                                 AF.Identity,
                                                 bias=bqk_sb[:, 1, dt:dt + 1])
                    for nt in range(NT):
                        v_ps = pbps.tile([128, 512], F32, tag="qkv_ps")
                        for ct in range(CT):
                            nc.tensor.matmul(
                                v_ps[:],
                                hT[:, ct, bass.ts(nt, 128)],
                                w_r["wv"][:, ct, :],
                                start=(ct == 0), stop=(ct == CT - 1))
                        v_sb = qko.tile([128, C], F32R, tag="v_sb")
                        nc.vector.tensor_tensor(v_sb[:], v_ps[:], bvb[:], op=ALU.add)
                        nc.sync.dma_start(v_dram[nt], v_sb[:])

            # ---- phase C: attention + proj + residual ----------------------
            with (
                tc.tile_pool(name="pc_s", bufs=2, space=bass.MemorySpace.PSUM) as pcs,
                tc.tile_pool(name="pc_av", bufs=CT, space=bass.MemorySpace.PSUM) as pcav,
                tc.tile_pool(name="pc_den", bufs=1, space=bass.MemorySpace.PSUM) as pcden,
                tc.tile_pool(name="pc_o", bufs=1, space=bass.MemorySpace.PSUM) as pco,
                tc.tile_pool(name="qgp", bufs=2) as qgp,
                tc.tile_pool(name="vst", bufs=4) as vst,
                tc.tile_pool(name="atp", bufs=3) as atp,
                tc.tile_pool(name="avtp", bufs=2) as avtp,
                tc.tile_pool(name="xbp", bufs=3) as xbp,
                tc.tile_pool(name="obp", bufs=3) as obp,
                tc.tile_pool(name="rp", bufs=2) as rp,
            ):
                for g in range(NG):
                    qg = qgp.tile([128, CT, GW], F32R, tag="qg")
                    for dt in range(CT):
                        nc.sync.dma_start(qg[:, dt, :],
                                          qt_dram[dt, :, bass.ts(g, GW)])
                    av_ps = [pcav.tile([128, GW], F32, tag="av", name=f"av_ps{dt}")
                             for dt in range(CT)]
                    den_ps = pcden.tile([1, GW], F32, tag="den")
                    for jt in range(NT):
                        s_ps = pcs.tile([128, GW], F32, tag="s_ps")
                        for ct in range(CT):
                            nc.tensor.matmul(
                                s_ps[:],
                                kT[:, ct, bass.ts(jt, 128)],
                                qg[:, ct, :],
                                start=(ct == 0), stop=(ct == CT - 1))
                        a_t = atp.tile([128, GW], F32R, tag="a_t")
                        nc.scalar.activation(a_t[:], s_ps[:], AF.Exp,
                                             scale=float(C) ** -0.5)
                        v_t = vst.tile([128, C], F32R, tag="v_t")
                        nc.sync.dma_start(v_t[:], v_dram[jt])
                        nc.tensor.matmul(den_ps[:], ones_r[:], a_t[:],
                                         start=(jt == 0), stop=(jt == NT - 1))
                        for dt in range(CT):
                            nc.tensor.matmul(
                                av_ps[dt][:],
                                v_t[:, bass.ts(dt, 128)],
                                a_t[:],
                                start=(jt == 0), stop=(jt == NT - 1))
                    # denominator -> reciprocal -> per-partition via DRAM bounce
                    recip = rp.tile([1, GW], F32, tag="recip")
                    nc.vector.reciprocal(recip[:], den_ps[:])
                    nc.sync.dma_start(den_bounce[g].unsqueeze(0), recip[:])
                    r_sb = rp.tile([128, NB], F32, tag="r_sb")
                    nc.sync.dma_start(
                        r_sb[:], den_bounce[g].rearrange("(nb p) -> p nb", p=128))
                    # AV^T -> SBUF (f32r) for proj lhsT
                    avT = avtp.tile([128, CT, GW], F32R, tag="avT")
                    for dt in range(CT):
                        nc.vector.tensor_copy(avT[:, dt, :], av_ps[dt][:])
                    for nb in range(NB):
                        o_ps = pco.tile([128, C], F32, tag="o_ps")
                        for dt in range(CT):
                            nc.tensor.matmul(
                                o_ps[:],
                                avT[:, dt, bass.ts(nb, 128)],
                                w_r["wo"][:, dt, :],
                                start=(dt == 0), stop=(dt == CT - 1))
                        nt = g * NB + nb
                        xb = xbp.tile([128, C], F32, tag="xb")
                        nc.sync.dma_start(xb[:], x_r[nt])
                        nc.vector.tensor_tensor(xb[:], xb[:], bob[:], op=ALU.add)
                        ob = obp.tile([128, C], F32, tag="ob")
                        nc.vector.scalar_tensor_tensor(
                            ob[:], o_ps[:], r_sb[:, nb:nb + 1], xb[:],
                            op0=ALU.mult, op1=ALU.add)
                        nc.sync.dma_start(out_r[nt], ob[:])

    nc.compile()
    return nc


_CACHE = {}


def _get_program(reps: int = 1):
    if reps not in _CACHE:
        _CACHE[reps] = build_program(reps)
    return _CACHE[reps]


def make_in_maps(inputs):
    ident = np.eye(128, dtype=np.float32)
    x = np.asarray(inputs["x"], dtype=np.float32).reshape(B, N, C)
    shared = {k: np.ascontiguousarray(np.asarray(inputs[k], dtype=np.float32))
              for k in ("wq", "wk", "wv", "wo", "bq", "bk", "bv", "bo",
                        "gn_scale", "gn_bias")}
    return [dict(x=np.ascontiguousarray(x[c]), ident=ident, **shared)
            for c in range(N_CORES)]


def kernel(**inputs) -> np.ndarray:
    nc = _get_program()
    in_maps = make_in_maps(inputs)
    res = run_bass_kernel_spmd(nc, in_maps, list(range(N_CORES)))
    out = np.stack([res.results[c]["out"] for c in range(N_CORES)], axis=0)
    return out.reshape(B, H, W, C)
